# revision 32
# baseline (speedup 1.0000x reference)
"""Two-layer GAT (N=4096, 4 heads, HID=256) on 8 TRN2 NeuronCores.

Sharding: each core owns N/8 = 512 destination rows of every N^2 attention
matrix. Weights are replicated. Per head we compute the local projection
g_shard = h_shard @ W.T on the owning core, then AllGather a packed
[512, 258] payload (g*B02 | B02 | B08) so every core has the full
[4096, 258] augmented g for the attention matmul.

Exp-space attention (exact rewrite, no N^2 transcendentals):
  exp(lrelu(u)) = e^{0.2u} * max(e^{0.8u}, 1)   for u = s_src_i + s_dst_j
The e^{0.8u} term is rank-1: A08_i * B08_j with A08 = e^{0.8 s_src},
B08 = e^{0.8 s_dst}. The e^{0.2 s_dst_j} factor is pre-multiplied into the
gathered payload (g' = B02 * g, denominator column = B02), and the
e^{0.2 s_src_i} factor is constant per destination row so it cancels in
the softmax normalization. Per attention tile the whole lrelu+exp chain
is then ONE DVE tensor_scalar (A08 * B08col, then max with 1.0 -- runs in
4x_2p mode) plus ONE batched mask multiply by the 0/1 adjacency (2x_1p),
optionally offloaded to the otherwise-idle GpSimd engine (style 'G').

Softmax needs no row reductions: the masked matrix P multiplies the
payload whose column 256 is B02, so the PSUM accumulator holds both the
numerator P@g' and the denominator P@B02; a per-partition
reciprocal-multiply normalizes after the matmul (the e^{-0.2 s_src}
factor common to both cancels exactly).

Layout: attention tiles are [j=source (partition), i=dest (free)], so P
tiles feed the PE matmul directly as lhsT with no transposes. The
gathered payload is DMAd in 4 quarter tiles so each group of attention
matmuls depends only on its own quarter, not the whole transfer.

All matmul operands are bf16 (fp32 matmuls run at 4 cycles/row on TRN2 vs
1 for bf16); accumulation stays fp32 in PSUM.

A dummy 1-tile AllGather is issued first so the ~45us collective
bootstrap barrier runs concurrently with the initial weight DMAs instead
of blocking the first real AllGather.
"""

import os

import numpy as np
import ml_dtypes

import concourse.bass as bass
import concourse.tile as tile
from concourse import bacc, mybir
from concourse.bass_utils import run_bass_kernel_spmd
from concourse.masks import make_identity

N, IN_DIM, HID, HEADS, OUT_DIM = 4096, 768, 256, 4, 32
ALPHA = 0.2
NCORES = 8
R = N // NCORES          # 512 rows per core
RB = R // 128            # 4 row blocks
FB = IN_DIM // 128       # 6 feature blocks
JB = N // 128            # 32 source chunks
HC = (HID * HEADS) // 128  # 8 concat-feature chunks
GW = HID + 2             # payload width: g*B02 (256) | B02 | B08
G = 8                    # jj tiles per elementwise group (= one gf quarter)
NG = JB // G             # 4 groups per head

F32 = mybir.dt.float32
BF16 = mybir.dt.bfloat16
AF = mybir.ActivationFunctionType
OP = mybir.AluOpType

last_exec_time_ns = None
_nc_cache = None

# elementwise style per group (T = max(A08_i * B08_j, 1), P = T * M):
#  'D': DVE tensor_scalar (mult B08col, max 1) per tile + DVE TT mask (2x)
#  'G': same tensor_scalar + GpSimd TT mask      (frees DVE; Pool idle)
#  'A': ACT Relu(B08col*A08 - 1) per tile [=T-1] + one batched DVE STT
#       (T' add 1) mult M  -- folds the mask multiply into the +1 fixup
#  'Q': ACT Relu per tile + batched DVE TS-imm +1 (single-src, 4x mode)
#       + GpSimd TT mask  -- cheapest DVE path (Pool can't run STT)
GRECIPE = list("QADG")
assert len(GRECIPE) == NG
HPACK = 2                # heads packed per AllGather


def _build_layer(nc, tc, pools, x_tiles, W_ap, WT_ap, ap_ap, mask_all, L,
                 after_phase_a=None):
    """One GAT layer. x_tiles: 6 SBUF tiles [128, R] bf16 (features x rows,
    feature-major). Returns 8 SBUF tiles [128, R] bf16 = concat-head
    activations transposed (x_gatT), elu applied."""
    sb = pools["sb"]
    ps_acc = pools["ps_acc"]
    ps_big = pools["ps_big"]
    ps_sm = pools["ps_sm"]
    dram_pay = pools["dram_pay"]
    dram_gat = pools["dram_gat"]
    ident = pools["ident"]

    groups = [list(range(NCORES))]

    head_state = []
    pair_gat = []
    # ---- Phase A: per-head projection + payload + AllGather ----
    for h in range(HEADS):
        # weights for this head
        W_t = []
        for cc in range(2):
            wt = sb.tile([128, IN_DIM], BF16, name=f"W_L{L}h{h}c{cc}", tag="Wh", bufs=4)
            nc.sync.dma_start(out=wt[:, :], in_=W_ap[h, cc * 128:(cc + 1) * 128, :])
            W_t.append(wt)
        WTaug = []
        for fb in range(FB):
            wta = sb.tile([128, HID + 2], BF16, name=f"WTa_L{L}h{h}f{fb}", tag="WTaug",
                          bufs=2 * FB)
            nc.sync.dma_start(out=wta[:, 0:HID], in_=WT_ap[h, fb * 128:(fb + 1) * 128, :])
            WTaug.append(wta)
        a_t = []
        for cc in range(2):
            at = sb.tile([128, 2], BF16, name=f"a_L{L}h{h}c{cc}", tag="ah", bufs=4)
            nc.sync.dma_start(out=at[:, :], in_=ap_ap[h, cc * 128:(cc + 1) * 128, :])
            a_t.append(at)

        # w_eff[f, 0:2] = W.T @ [a_src | a_dst]  -> [768, 2] in 6 blocks
        weff = []
        for fb in range(FB):
            pw = ps_sm.tile([128, 2], F32, name=f"pw_L{L}h{h}f{fb}", tag="ps_sm")
            for cc in range(2):
                nc.tensor.matmul(pw[:, :], lhsT=W_t[cc][:, fb * 128:(fb + 1) * 128],
                                 rhs=a_t[cc][:, :], start=(cc == 0), stop=(cc == 1))
            wf = sb.tile([128, 2], BF16, name=f"weff_L{L}h{h}f{fb}", tag="weff",
                         bufs=2 * FB)
            nc.vector.tensor_copy(wf[:, :], pw[:, :])
            # dst/src halves become columns HID/HID+1 of the projection
            # rhs, so pg yields s_dst AND s_src as free extra columns
            nc.vector.tensor_copy(WTaug[fb][:, HID:HID + 1], wf[:, 1:2])
            nc.vector.tensor_copy(WTaug[fb][:, HID + 1:HID + 2], wf[:, 0:1])
            weff.append(wf)

        # g_aug = x.T @ WTaug -> [512, 258] (g | s_dst | s_src) in PSUM;
        # payload packs (g*B02 | B02 | B08) with B0x = exp(0.x * s_dst).
        # HPACK heads share one payload tensor (column-blocked) so the
        # AllGather count halves; the bootstrap/skew constant amortizes.
        if h % HPACK == 0:
            pay_t = dram_pay.tile([R, HPACK * GW], BF16, name=f"pay_L{L}p{h//HPACK}",
                                  tag="pay")
            pools["cur_pay"] = pay_t
        else:
            pay_t = pools["cur_pay"]
        hoff = (h % HPACK) * GW
        pl = sb.tile([128, RB * GW], BF16, name=f"pl_L{L}h{h}", tag="pl", bufs=2)
        ssc = sb.tile([128, RB], BF16, name=f"ssc_L{L}h{h}", tag="ssc", bufs=2)
        for ib in range(RB):
            pg = ps_big.tile([128, HID + 2], F32, name=f"pg_L{L}h{h}b{ib}", tag="ps_big")
            for fb in range(FB):
                nc.tensor.matmul(pg[:, :], lhsT=x_tiles[fb][:, ib * 128:(ib + 1) * 128],
                                 rhs=WTaug[fb][:, :], start=(fb == 0),
                                 stop=(fb == FB - 1))
            o = ib * GW
            b02 = sb.tile([128, 1], F32, name=f"b02_L{L}h{h}b{ib}", tag="b02", bufs=4)
            nc.scalar.activation(b02[:, :], pg[:, HID:HID + 1], AF.Exp, scale=0.2)
            # g' = g * B02 (per-partition scale) and bf16 cast, on ACT
            nc.scalar.activation(pl[:, o:o + HID], pg[:, 0:HID], AF.Copy,
                                 scale=b02[:, 0:1])
            nc.vector.tensor_copy(pl[:, o + HID:o + HID + 1], b02[:, 0:1])
            # B08 = B02^4 (avoids a second PSUM-read Exp on ACT)
            b04 = sb.tile([128, 1], F32, name=f"b04_L{L}h{h}b{ib}", tag="b02", bufs=4)
            nc.vector.tensor_tensor(b04[:, :], b02[:, :], b02[:, :], OP.mult)
            nc.vector.tensor_tensor(pl[:, o + HID + 1:o + HID + 2], b04[:, :],
                                    b04[:, :], OP.mult)
            nc.vector.tensor_copy(ssc[:, ib:ib + 1], pg[:, HID + 1:HID + 2])
        # s_src broadcast from the collected columns: one transpose + 4
        # one-hot selector matmuls; then A08 = exp(0.8 * s_src) on ACT
        pst = ps_sm.tile([RB, 128], BF16, name=f"pst_L{L}h{h}", tag="ps_sm")
        nc.tensor.transpose(pst[:, :], ssc[:, :], ident[:, :])
        sscT = sb.tile([RB, 128], BF16, name=f"sscT_L{L}h{h}", tag="sscT", bufs=2)
        nc.vector.tensor_copy(sscT[:, :], pst[:, :])
        sel = pools["sel"]
        pb = ps_big.tile([128, R], F32, name=f"pb_L{L}h{h}", tag="ps_big")
        for ib in range(RB):
            nc.tensor.matmul(pb[:, ib * 128:(ib + 1) * 128],
                             lhsT=sel[:, ib * 128:(ib + 1) * 128],
                             rhs=sscT[:, :], start=True, stop=True)
        a08 = sb.tile([128, R], BF16, name=f"a08_L{L}h{h}", tag="a08", bufs=2)
        nc.scalar.activation(a08[:, :], pb[:, :], AF.Exp, scale=0.8)
        # second copy of A08 in a different SBUF region: the DVE-side
        # per-tile ops read this one while ACT Relus read a08, halving
        # SBUF port contention on the shared broadcast tile
        a08b = sb.tile([128, R], BF16, name=f"a08b_L{L}h{h}", tag="a08b", bufs=2)
        nc.vector.tensor_copy(a08b[:, :], a08[:, :])
        # one DMA: SBUF [p, (ib, c)] -> DRAM [(ib, p), c-block of this head]
        nc.sync.dma_start(
            out=pay_t.rearrange("(ib p) c -> p ib c", p=128)[:, :, hoff:hoff + GW],
            in_=pl.rearrange("p (ib c) -> p ib c", c=GW))

        if h % HPACK == HPACK - 1:
            gat_t = dram_gat.tile([N, HPACK * GW], BF16, name=f"gat_L{L}p{h//HPACK}",
                                  tag="gat", addr_space="Shared")
            nc.gpsimd.collective_compute(
                "AllGather", OP.bypass, replica_groups=groups,
                ins=[pay_t.opt()], outs=[gat_t.opt()],
            )
            pair_gat.append(gat_t)
        head_state.append((h // HPACK, hoff, a08, a08b))

    if after_phase_a is not None:
        after_phase_a()

    # ---- Phase B: attention per head ----
    xgatT = []
    for hc in range(HC):
        xg = sb.tile([128, R], BF16, name=f"xgatT_L{L}c{hc}", tag="xgatT", bufs=HC)
        xgatT.append(xg)

    GW2 = HPACK * GW
    pair_gfq = {}
    for h in range(HEADS):
        pr, hoff, a08, a08b = head_state[h]
        gat_t = pair_gat[pr]
        # quarter-granularity DMA per head-pair: each group of matmuls
        # depends only on its own quarter of the gathered payload
        if pr not in pair_gfq:
            gfq = []
            for qt in range(4):
                js = slice(qt * JB // 4, (qt + 1) * JB // 4)
                gq = sb.tile([128, (JB // 4) * GW2], BF16, name=f"gf_L{L}p{pr}q{qt}",
                             tag="gfq", bufs=5)
                nc.sync.dma_start(
                    out=gq.rearrange("p (j c) -> p j c", c=GW2)[:, :],
                    in_=gat_t.rearrange("(j p) c -> p j c", p=128)[:, js])
                gfq.append(gq)
            pair_gfq[pr] = gfq
        gfq = pair_gfq[pr]
        # f32 B08 scalar columns per quarter (scalar/scale APs must be f32)
        sdq = []
        for qt in range(4):
            sq = sb.tile([128, JB // 4], F32, name=f"sd_L{L}h{h}q{qt}",
                         tag="sdq", bufs=8)
            nc.vector.tensor_copy(
                sq[:, :],
                gfq[qt].rearrange("p (j c) -> p j c", c=GW2)[:, :, hoff + GW - 1])
            sdq.append(sq)

        def b08col(qt, j):
            return sdq[qt][:, j:j + 1]

        U = []
        for ib in range(RB):
            u = ps_acc.tile([128, HID + 1], F32, name=f"U_L{L}h{h}b{ib}", tag="ps_acc")
            U.append(u)

        for g in range(NG):
            qt = (g * G) // (JB // 4)
            gbase = (g * G) % (JB // 4)   # jj offset within the quarter
            style = GRECIPE[g]
            ubig = sb.tile([128, G * R], BF16, name=f"ub_L{L}h{h}g{g}", tag="ubig",
                           bufs=2)
            pmbig = sb.tile([128, G * R], BF16, name=f"pm_L{L}h{h}g{g}", tag="pmbig",
                            bufs=3)
            if style in ("A", "Q"):
                # T-1 = Relu(B08col * A08 - 1) per tile on ACT
                neg1 = pools["neg1"]
                for q in range(G):
                    nc.scalar.activation(
                        ubig[:, q * R:(q + 1) * R], a08[:, :], AF.Relu,
                        bias=neg1[:, 0:1], scale=b08col(qt, gbase + q))
                if style == "A":
                    # one batched STT: P = (T-1 add 1) mult M
                    nc.vector.scalar_tensor_tensor(
                        pmbig[:, :], ubig[:, :], 1.0,
                        mask_all[:, g * G * R:(g + 1) * G * R], OP.add, OP.mult)
                else:
                    # batched single-src +1 on DVE (4x mode), mask on Pool
                    vbig = sb.tile([128, G * R], BF16, name=f"vb_L{L}h{h}g{g}",
                                   tag="vbig", bufs=2)
                    nc.vector.tensor_scalar(vbig[:, :], ubig[:, :], 1.0, None,
                                            OP.add)
                    nc.gpsimd.tensor_tensor(
                        pmbig[:, :], vbig[:, :],
                        mask_all[:, g * G * R:(g + 1) * G * R], OP.mult)
            else:
                # T = max(A08*B08col, 1) per tile on DVE, then batched TT mask
                for q in range(G):
                    nc.vector.tensor_scalar(
                        ubig[:, q * R:(q + 1) * R], a08b[:, :],
                        sdq[qt][:, gbase + q:gbase + q + 1], 1.0, OP.mult, OP.max)
                eng = nc.gpsimd if style == "G" else nc.vector
                eng.tensor_tensor(pmbig[:, :], ubig[:, :],
                                  mask_all[:, g * G * R:(g + 1) * G * R], OP.mult)
            for q in range(G):
                jj = g * G + q
                o = (gbase + q) * GW2 + hoff
                for ib in range(RB):
                    nc.tensor.matmul(
                        U[ib][:, :],
                        lhsT=pmbig[:, q * R + ib * 128:q * R + (ib + 1) * 128],
                        rhs=gfq[qt][:, o:o + HID + 1], start=(jj == 0),
                        stop=(jj == JB - 1))

        for ib in range(RB):
            rcp = sb.tile([128, 1], F32, name=f"rcp_L{L}h{h}b{ib}", tag="rcp", bufs=2)
            nc.vector.reciprocal(rcp[:, :], U[ib][:, HID:HID + 1])
            # hn = U / denom on ACT (per-partition scale), bf16 out
            hn = sb.tile([128, HID], BF16, name=f"hn_L{L}h{h}b{ib}", tag="hn", bufs=2)
            nc.scalar.activation(hn[:, :], U[ib][:, 0:HID], AF.Copy,
                                 scale=rcp[:, 0:1])
            # elu(x) = max(x, exp(min(x, 0)) - 1)
            t1 = sb.tile([128, HID], BF16, name=f"t1_L{L}h{h}b{ib}", tag="t1", bufs=2)
            nc.vector.tensor_scalar_min(t1[:, :], hn[:, :], 0.0)
            t2 = sb.tile([128, HID], BF16, name=f"t2_L{L}h{h}b{ib}", tag="t2", bufs=2)
            nc.scalar.activation(t2[:, :], t1[:, :], AF.Exp)
            eo = sb.tile([128, HID], BF16, name=f"eo_L{L}h{h}b{ib}", tag="eo", bufs=2)
            nc.vector.scalar_tensor_tensor(eo[:, :], t2[:, :], -1.0, hn[:, :],
                                           OP.add, OP.max)
            # transpose via the DMA crossbar: no PE, no PSUM bank, no copy-back
            for cb in range(2):
                nc.sync.dma_start_transpose(
                    xgatT[h * 2 + cb][:, ib * 128:(ib + 1) * 128],
                    eo[:, cb * 128:(cb + 1) * 128])
    return xgatT


def _build_program():
    nc = bacc.Bacc("TRN2", target_bir_lowering=False, debug=False,
                   num_devices=NCORES)

    xT_in = nc.dram_tensor("xT", [IN_DIM, R], BF16, kind="ExternalInput").ap()
    sel_in = nc.dram_tensor("selhot", [RB, RB * 128], BF16, kind="ExternalInput").ap()
    mask_in = nc.dram_tensor("madd", [N, R], BF16, kind="ExternalInput").ap()
    W1_in = nc.dram_tensor("W1", [HEADS, HID, IN_DIM], BF16, kind="ExternalInput").ap()
    W1T_in = nc.dram_tensor("W1T", [HEADS, IN_DIM, HID], BF16, kind="ExternalInput").ap()
    a1_in = nc.dram_tensor("a1p", [HEADS, HID, 2], BF16, kind="ExternalInput").ap()
    W2_in = nc.dram_tensor("W2", [HEADS, HID, IN_DIM], BF16, kind="ExternalInput").ap()
    W2T_in = nc.dram_tensor("W2T", [HEADS, IN_DIM, HID], BF16, kind="ExternalInput").ap()
    a2_in = nc.dram_tensor("a2p", [HEADS, HID, 2], BF16, kind="ExternalInput").ap()
    outwT_in = nc.dram_tensor("outwT", [HID * HEADS, IN_DIM], BF16,
                              kind="ExternalInput").ap()
    outb_in = nc.dram_tensor("outb", [IN_DIM, 1], F32, kind="ExternalInput").ap()
    out2wT_in = nc.dram_tensor("out2wT", [HID * HEADS, OUT_DIM], BF16,
                               kind="ExternalInput").ap()
    out2b_in = nc.dram_tensor("out2b", [OUT_DIM, 1], F32, kind="ExternalInput").ap()
    outT = nc.dram_tensor("outT", [OUT_DIM, R], F32, kind="ExternalOutput").ap()

    groups = [list(range(NCORES))]

    with tile.TileContext(nc) as tc:
        with tc.tile_pool(name="sb", bufs=1) as sb, \
             tc.tile_pool(name="ps_acc", bufs=5, space="PSUM") as ps_acc, \
             tc.tile_pool(name="ps_big", bufs=2, space="PSUM") as ps_big, \
             tc.tile_pool(name="ps_sm", bufs=1, space="PSUM") as ps_sm, \
             tc.tile_pool(name="dram_pay", bufs=4, space="DRAM") as dram_pay, \
             tc.tile_pool(name="dram_gat", bufs=3, space="DRAM") as dram_gat:

            pools = dict(sb=sb, ps_acc=ps_acc, ps_big=ps_big, ps_sm=ps_sm,
                         dram_pay=dram_pay, dram_gat=dram_gat)

            # dummy 1-tile AllGather issued first: ties the collective
            # bootstrap barrier to an op with no input dependency so it
            # completes while the initial DMAs run
            dummy_in = dram_pay.tile([1, 16], BF16, name="dummy_in", tag="dummy_i")
            dummy_out = dram_gat.tile([NCORES, 16], BF16, name="dummy_out",
                                      tag="dummy_o", addr_space="Shared")
            nc.gpsimd.collective_compute(
                "AllGather", OP.bypass, replica_groups=groups,
                ins=[dummy_in.opt()], outs=[dummy_out.opt()],
            )

            # constants
            ident = sb.tile([128, 128], BF16, name="ident", tag="ident", bufs=1)
            make_identity(nc, ident[:, :])
            # sel[k, ib*128+p] = (k == ib): one-hot rows for the s_src
            # broadcast matmuls (host-supplied; Memset can't write at a
            # non-zero base partition)
            sel = sb.tile([RB, RB * 128], BF16, name="sel", tag="sel", bufs=1)
            nc.sync.dma_start(out=sel[:, :], in_=sel_in[:, :])
            neg1 = sb.tile([128, 1], F32, name="neg1", tag="neg1", bufs=1)
            nc.vector.memset(neg1[:, :], -1.0)
            pools["ident"] = ident
            pools["sel"] = sel
            pools["neg1"] = neg1

            # resident inputs needed for L1 phase A
            x0 = []
            for fb in range(FB):
                x = sb.tile([128, R], BF16, name=f"x0_{fb}", tag="x0", bufs=FB)
                nc.sync.dma_start(out=x[:, :], in_=xT_in[fb * 128:(fb + 1) * 128, :])
                x0.append(x)

            # adjacency mask (0/1 bf16), loaded after L1 phase A is issued
            # so it doesn't delay the W1/x DMAs it shares a queue with
            mask_all = sb.tile([128, JB * R], BF16, name="mask_all", tag="madd",
                               bufs=1)
            outw_t = []
            outb_t = []
            out2w_t = []

            def load_l1_deferred():
                for q in range(4):
                    js = slice(q * JB // 4, (q + 1) * JB // 4)
                    nc.sync.dma_start(
                        out=mask_all.rearrange("p (j c) -> p j c", c=R)[:, js],
                        in_=mask_in.rearrange("(j p) c -> p j c", p=128)[:, js])
                for hc in range(HC):
                    w = sb.tile([128, IN_DIM], BF16, name=f"outw{hc}", tag="outw",
                                bufs=HC)
                    nc.sync.dma_start(out=w[:, :],
                                      in_=outwT_in[hc * 128:(hc + 1) * 128, :])
                    outw_t.append(w)
                for fb in range(FB):
                    b = sb.tile([128, 1], F32, name=f"outb{fb}", tag="outb", bufs=FB)
                    nc.sync.dma_start(out=b[:, :],
                                      in_=outb_in[fb * 128:(fb + 1) * 128, :])
                    outb_t.append(b)
                for hc in range(HC):
                    w = sb.tile([128, OUT_DIM], BF16, name=f"out2w{hc}", tag="out2w",
                                bufs=HC)
                    nc.sync.dma_start(out=w[:, :],
                                      in_=out2wT_in[hc * 128:(hc + 1) * 128, :])
                    out2w_t.append(w)

            # ---- layer 1 ----
            xg1 = _build_layer(nc, tc, pools, x0, W1_in, W1T_in, a1_in, mask_all, 1,
                               after_phase_a=load_l1_deferred)
            out2b_t = sb.tile([OUT_DIM, 1], F32, name="out2b", tag="out2b", bufs=1)
            nc.sync.dma_start(out=out2b_t[:, :], in_=out2b_in[:, :])
            x1 = []
            for fb in range(FB):
                px = ps_acc.tile([128, R], F32, name=f"px1_{fb}", tag="ps_acc")
                for hc in range(HC):
                    nc.tensor.matmul(px[:, :], lhsT=outw_t[hc][:, fb * 128:(fb + 1) * 128],
                                     rhs=xg1[hc][:, :], start=(hc == 0),
                                     stop=(hc == HC - 1))
                x = sb.tile([128, R], BF16, name=f"x1_{fb}", tag="x1", bufs=FB)
                nc.scalar.activation(x[:, :], px[:, :], AF.Identity,
                                     bias=outb_t[fb][:, 0:1])
                x1.append(x)

            # ---- layer 2 ----
            xg2 = _build_layer(nc, tc, pools, x1, W2_in, W2T_in, a2_in, mask_all, 2)
            po = ps_big.tile([OUT_DIM, R], F32, name="po", tag="ps_big")
            for hc in range(HC):
                nc.tensor.matmul(po[:, :], lhsT=out2w_t[hc][:, 0:OUT_DIM],
                                 rhs=xg2[hc][:, :], start=(hc == 0),
                                 stop=(hc == HC - 1))
            ot = sb.tile([OUT_DIM, R], F32, name="ot", tag="ot", bufs=1)
            nc.vector.tensor_scalar(ot[:, :], po[:, :], out2b_t[:, 0:1], None, OP.add)
            nc.sync.dma_start(out=outT[:, :], in_=ot[:, :])

    nc.compile()
    return nc


def _host_shards(label_mat, W1, a1, W2, a2, out_w, out_b, out2_w, out2_b, adj):
    f32 = np.float32
    bf16 = ml_dtypes.bfloat16
    label_T = np.asarray(label_mat, f32).T.astype(bf16)                 # [768, N]
    adjT_01 = (np.asarray(adj).T != 0).astype(bf16)                     # 1 / 0
    common = dict(
        W1=np.ascontiguousarray(np.asarray(W1, f32).astype(bf16)),
        W1T=np.ascontiguousarray(np.asarray(W1, f32).transpose(0, 2, 1).astype(bf16)),
        a1p=np.ascontiguousarray(np.asarray(a1, f32).reshape(HEADS, 2, HID)
                                 .transpose(0, 2, 1).astype(bf16)),
        W2=np.ascontiguousarray(np.asarray(W2, f32).astype(bf16)),
        W2T=np.ascontiguousarray(np.asarray(W2, f32).transpose(0, 2, 1).astype(bf16)),
        a2p=np.ascontiguousarray(np.asarray(a2, f32).reshape(HEADS, 2, HID)
                                 .transpose(0, 2, 1).astype(bf16)),
        outwT=np.ascontiguousarray(np.asarray(out_w, f32).T.astype(bf16)),
        outb=np.ascontiguousarray(np.asarray(out_b, f32).reshape(IN_DIM, 1)),
        out2wT=np.ascontiguousarray(np.asarray(out2_w, f32).T.astype(bf16)),
        out2b=np.ascontiguousarray(np.asarray(out2_b, f32).reshape(OUT_DIM, 1)),
        selhot=np.ascontiguousarray(
            np.kron(np.eye(R // 128, dtype=f32), np.ones((1, 128), f32))
            .astype(bf16)),
    )
    in_maps = []
    for c in range(NCORES):
        sl = slice(c * R, (c + 1) * R)
        m = dict(common)
        m["xT"] = np.ascontiguousarray(label_T[:, sl])
        m["madd"] = np.ascontiguousarray(adjT_01[:, sl])
        in_maps.append(m)
    return in_maps


def kernel(**inputs):
    global _nc_cache, last_exec_time_ns
    if _nc_cache is None:
        _nc_cache = _build_program()
    nc = _nc_cache
    in_maps = _host_shards(**inputs)
    trace = os.environ.get("GAT_TRACE", "0") == "1"
    res = run_bass_kernel_spmd(nc, in_maps, list(range(NCORES)), trace=trace)
    last_exec_time_ns = res.exec_time_ns
    out = np.empty((N, OUT_DIM), np.float32)
    for c in range(NCORES):
        out[c * R:(c + 1) * R, :] = np.asarray(res.results[c]["outT"]).T
    return out
